# revision 1
# baseline (speedup 1.0000x reference)
"""TRN2 Bass kernel for nn_Attention_59081570125142.

MobileViT-style attention block:
  qkv = BN(1x1conv(x)); per-head attention over N=1024 tokens
  (key_dim=16, head_dim=32, 8 heads); pos_enc = BN(dwconv3x3(v));
  out = BN(1x1conv(v_attn + pos_enc)).

Sharding: data-parallel over batch B=16 across 8 cores (2 images/core).
All BN folded into conv weights/biases on host. Matmuls in bf16
(measured end-to-end rel err ~3e-3), accumulation fp32 in PSUM.

Softmax layout trick: logits computed KEY-major (logitsT[k, q] via
lhsT=k_head, rhs=q_head) so softmax normalizer is a column sum that
rides for free as a ones-column appended to v^T in the AV matmul
(out rows 0..31 = unnormalized AV, row 32 = denominator).
QK packs 4 heads in the PE array via row tiling (K=16 each);
AV packs 2 heads via col tiling (M=33 at psum partition 0/64).
"""
import sys

sys.path.insert(0, '/opt/trn_rl_repo')

import numpy as np
import ml_dtypes

import concourse.bass as bass
import concourse.mybir as mybir
from concourse import tile
from concourse.bass_utils import run_bass_kernel_spmd

F32 = mybir.dt.float32
F32R = mybir.dt.float32r
BF16 = mybir.dt.bfloat16
AF = mybir.ActivationFunctionType
ALU = mybir.AluOpType

N_CORES = 8
B = 16
B_LOC = B // N_CORES          # 2 images per core
DIM = 256
NUM_HEADS = 8
HEAD_DIM = 32
KEY_DIM = 16
QKV_OUT = 512
N = 1024                      # 32*32 tokens
H = W = 32
SCALE = KEY_DIM ** -0.5       # 0.25
BN_EPS = 1e-3

_cache = {}


def _fold_bn(w, gamma, beta, mean, var):
    inv = gamma / np.sqrt(var + BN_EPS)
    return w * inv[:, None], beta - mean * inv


def _prep_weights(qkv_w, qkv_gamma, qkv_beta, qkv_mean, qkv_var,
                  pe_w, pe_gamma, pe_beta, pe_mean, pe_var,
                  proj_w, proj_gamma, proj_beta, proj_mean, proj_var):
    """Host-side BN folding + layout rearrangement."""
    W1, b1 = _fold_bn(qkv_w[:, :, 0, 0].astype(np.float32), qkv_gamma, qkv_beta,
                      qkv_mean, qkv_var)                      # (512, 256), (512,)
    W2, b2 = _fold_bn(proj_w[:, :, 0, 0].astype(np.float32), proj_gamma, proj_beta,
                      proj_mean, proj_var)                    # (256, 256), (256,)
    invpe = pe_gamma / np.sqrt(pe_var + BN_EPS)
    PW = (pe_w[:, 0] * invpe[:, None, None]).astype(np.float32)  # (256, 3, 3)
    bpe = (pe_beta - pe_mean * invpe).astype(np.float32)         # (256,)

    # channel-major qkv pass: out = W1T_cm.T @ x, 6 output tiles of 128:
    #  t0/t1: K heads 0-3 / 4-7 at 32-aligned slots (16 rows used per head)
    #  t2/t3: Q likewise;  t4/t5: V natural order (head h -> rows 32h..32h+32)
    w1t_cm = np.zeros((256, 768), np.float32)
    b1_cm = np.zeros((768,), np.float32)
    for h in range(NUM_HEADS):
        t = h // 4
        j = h % 4
        # K
        cols = t * 128 + 32 * j + np.arange(16)
        chans = h * 64 + 16 + np.arange(16)
        w1t_cm[:, cols] = W1[chans].T
        b1_cm[cols] = b1[chans]
        # Q
        cols = 256 + t * 128 + 32 * j + np.arange(16)
        chans = h * 64 + np.arange(16)
        w1t_cm[:, cols] = W1[chans].T
        b1_cm[cols] = b1[chans]
        # V
        cols = 512 + h * 32 + np.arange(32)
        chans = h * 64 + 32 + np.arange(32)
        w1t_cm[:, cols] = W1[chans].T
        b1_cm[cols] = b1[chans]

    # token-major v pass: vT[tok, 33h+d] = sum_c x[c,tok] * W1'[h*64+32+d, c];
    # col 33h+32 gets 0 from the matmul and 1.0 from the bias tile (ones col
    # for the fused softmax denominator).
    wvt = np.zeros((256, 264), np.float32)
    bvb = np.zeros((264,), np.float32)
    for h in range(NUM_HEADS):
        cols = 33 * h + np.arange(32)
        chans = h * 64 + 32 + np.arange(32)
        wvt[:, cols] = W1[chans].T
        bvb[cols] = b1[chans]
        bvb[33 * h + 32] = 1.0

    bf = ml_dtypes.bfloat16
    return dict(
        w1t=w1t_cm.astype(bf),                  # (256, 768)
        b1=b1_cm.reshape(768, 1).astype(np.float32),
        wvt=wvt.astype(bf),                     # (256, 264)
        bvb=np.broadcast_to(bvb, (128, 264)).copy().astype(np.float32),
        w2t=W2.T.copy().astype(bf),             # (256, 256)
        b2=b2.reshape(256, 1).astype(np.float32),
        pew=PW.reshape(256, 9).astype(np.float32),
        bpe=bpe.reshape(256, 1).astype(np.float32),
    )


def _split_waits(nc, cap=1, caps=None):
    """Walrus rejects instructions with more than ~1-2 semaphore waits
    ("Too many sync wait commands"). Hoist excess waits onto same-engine
    EventSemaphore carrier instructions inserted just before the offender —
    the engine stalls at the carriers first, semantically identical."""
    n = 0
    caps = caps or {}
    for f in nc.m.functions:
        for blk in f.blocks:
            insts = blk.instructions
            out = []
            for inst in insts:
                si = inst.sync_info
                waits = list(si.on_wait) if si else []
                # ACT executes strictly in order, so a wait on its OWN engine
                # semaphore (WAW guard for est-slot reuse) is redundant — but
                # it forces each exp to eat the previous one's write-ack
                # latency. Drop those.
                if (str(inst.engine) == "EngineType.Activation"
                        and waits and inst.opcode == "Activation"):
                    kept = [w for w in waits
                            if not str(w.ant_name).startswith("Activation")]
                    if len(kept) != len(waits):
                        waits = kept
                        si = mybir.SyncInfo(on_wait=waits,
                                            on_update=list(si.on_update))
                        inst.sync_info = si
                icap = caps.get(str(inst.opcode), cap)
                if len(waits) > icap and inst.opcode != "EventSemaphore":
                    extra, keep = waits[:-icap], waits[-icap:]
                    for k, w in enumerate(extra):
                        ev = mybir.InstEventSemaphore(
                            name=f"{inst.name}-sw{k}", ins=[], outs=[],
                            sync_info=mybir.SyncInfo(on_wait=[w], on_update=[]),
                        )
                        ev.engine = inst.engine
                        out.append(ev)
                        n += 1
                    inst.sync_info = mybir.SyncInfo(
                        on_wait=keep, on_update=list(si.on_update))
                out.append(inst)
            blk.instructions = out
    return n


def _build_program(reps=1):
    nc = bass.Bass()
    x_d = nc.declare_dram_parameter("x", [B_LOC, 256, N], BF16, isOutput=False)
    w1t_d = nc.declare_dram_parameter("w1t", [256, 768], BF16, isOutput=False)
    b1_d = nc.declare_dram_parameter("b1", [768, 1], F32, isOutput=False)
    wvt_d = nc.declare_dram_parameter("wvt", [256, 264], BF16, isOutput=False)
    bvb_d = nc.declare_dram_parameter("bvb", [128, 264], F32, isOutput=False)
    w2t_d = nc.declare_dram_parameter("w2t", [256, 256], BF16, isOutput=False)
    b2_d = nc.declare_dram_parameter("b2", [256, 1], F32, isOutput=False)
    pew_d = nc.declare_dram_parameter("pew", [256, 9], F32, isOutput=False)
    bpe_d = nc.declare_dram_parameter("bpe", [256, 1], F32, isOutput=False)
    out_d = nc.declare_dram_parameter("out", [B_LOC, 256, N], F32, isOutput=True)

    with tile.TileContext(nc) as tc:
        with (
            tc.tile_pool(name="const", bufs=1) as cpool,
            tc.tile_pool(name="xp", bufs=1) as xpool,
            tc.tile_pool(name="qkv", bufs=1) as qkvpool,
            tc.tile_pool(name="vt", bufs=1) as vtpool,
            tc.tile_pool(name="pe", bufs=1) as pepool,
            tc.tile_pool(name="petmp", bufs=3) as petmp,
            tc.tile_pool(name="exp", bufs=6) as exppool,
            tc.tile_pool(name="attn", bufs=1) as attnpool,
            tc.tile_pool(name="nrm", bufs=3) as nrmpool,
            tc.tile_pool(name="outp", bufs=4) as outpool,
            tc.tile_pool(name="ps_big", bufs=1, space="PSUM") as ps_big,     # 1 bank
            tc.tile_pool(name="ps_small", bufs=1, space="PSUM") as ps_small, # 1 bank (vt + bcast)
            tc.tile_pool(name="ps_qk", bufs=2, space="PSUM") as ps_qk,       # 4 banks
            tc.tile_pool(name="ps_av", bufs=2, space="PSUM") as ps_av,       # 2 banks
        ):
            # ---- load constants ----
            w1t = [cpool.tile([128, 768], BF16, tag=f"w1t{i}", name=f"w1t{i}") for i in range(2)]
            wvt = [cpool.tile([128, 264], BF16, tag=f"wvt{i}", name=f"wvt{i}") for i in range(2)]
            w2t = [cpool.tile([128, 256], BF16, tag=f"w2t{i}", name=f"w2t{i}") for i in range(2)]
            b2 = [cpool.tile([128, 1], F32, tag=f"b2{i}", name=f"b2{i}") for i in range(2)]
            pew = [cpool.tile([128, 9], F32, tag=f"pew{i}", name=f"pew{i}") for i in range(2)]
            bpe = [cpool.tile([128, 1], F32, tag=f"bpe{i}", name=f"bpe{i}") for i in range(2)]
            b1 = [cpool.tile([128, 1], F32, tag=f"b1{i}", name=f"b1{i}") for i in range(6)]
            bvb = cpool.tile([128, 264], F32, tag="bvb")
            for ct in range(2):
                nc.sync.dma_start(w1t[ct][:], w1t_d[128 * ct:128 * (ct + 1), :])
                nc.sync.dma_start(wvt[ct][:], wvt_d[128 * ct:128 * (ct + 1), :])
                nc.sync.dma_start(w2t[ct][:], w2t_d[128 * ct:128 * (ct + 1), :])
                nc.sync.dma_start(b2[ct][:], b2_d[128 * ct:128 * (ct + 1), :])
                nc.sync.dma_start(pew[ct][:], pew_d[128 * ct:128 * (ct + 1), :])
                nc.sync.dma_start(bpe[ct][:], bpe_d[128 * ct:128 * (ct + 1), :])
            for ot in range(6):
                nc.sync.dma_start(b1[ot][:], b1_d[128 * ot:128 * (ot + 1), :])
            nc.sync.dma_start(bvb[:], bvb_d[:])
            ones32 = cpool.tile([1, 32], BF16, tag="ones32", name="ones32")
            nc.vector.memset(ones32[:], 1.0)

            xs = [[xpool.tile([128, N], BF16, tag=f"x{im}{ct}", name=f"x{im}{ct}") for ct in range(2)]
                  for im in range(B_LOC)]
            for im in range(B_LOC):
                for ct in range(2):
                    nc.sync.dma_start(xs[im][ct][:], x_d[im, 128 * ct:128 * (ct + 1), :])

            for _rep in range(reps):
                qkv_all = [[qkvpool.tile([128, N], BF16, tag=f"qkv{im}{ot}",
                                         name=f"qkv{im}{ot}") for ot in range(6)]
                           for im in range(B_LOC)]
                vts_all = [[vtpool.tile([128, 264], BF16, tag=f"vt{im}{tt}",
                                        name=f"vt{im}{tt}") for tt in range(8)]
                           for im in range(B_LOC)]
                attn_all = [[attnpool.tile([128, N], BF16, tag=f"attn{im}{ct}",
                                           name=f"attn{im}{ct}")
                             for ct in range(2)] for im in range(B_LOC)]
                peacc_all = [[None, None] for _ in range(B_LOC)]

                def emit_qkv_unit(im, ot, nq):
                    # N=256 chunks keep per-filler-slot PE load under the
                    # ACT period so injected preproc never stalls the pipeline
                    pq = ps_big.tile([128, 256], F32, tag="big", name="pq")
                    for ct in range(2):
                        nc.tensor.matmul(
                            pq[:], w1t[ct][:, 128 * ot:128 * (ot + 1)],
                            xs[im][ct][:, 256 * nq:256 * (nq + 1)],
                            start=(ct == 0), stop=(ct == 1))
                    nc.vector.tensor_scalar_add(
                        qkv_all[im][ot][:, 256 * nq:256 * (nq + 1)],
                        pq[:], b1[ot][:])

                def emit_vt_unit(im, tt):
                    pv = ps_small.tile([128, 512], F32, tag="small", name="pv")
                    for ct in range(2):
                        nc.tensor.matmul(
                            pv[:, 0:264],
                            xs[im][ct][:, 128 * tt:128 * (tt + 1)], wvt[ct][:],
                            start=(ct == 0), stop=(ct == 1))
                    nc.vector.tensor_tensor(vts_all[im][tt][:], pv[:, 0:264],
                                            bvb[:], op=ALU.add)

                def dwconv_ops(im):
                    # depthwise 3x3 on v (bf16): list of single-op closures
                    ops = []
                    for ct in range(2):
                        vpad = petmp.tile([128, 34, 34], BF16, tag="vpad",
                                          name="vpad")
                        acc_a = pepool.tile([128, N], BF16,
                                            tag=f"peacc{im}{ct}",
                                            name=f"peacc{im}{ct}")
                        acc_b = petmp.tile([128, N], BF16, tag="peacc_tmp",
                                           name="peacc_tmp")
                        a2d = acc_a[:].rearrange("p (h w) -> p h w", h=H)
                        b2d = acc_b[:].rearrange("p (h w) -> p h w", h=H)
                        vimg = qkv_all[im][4 + ct][:].rearrange(
                            "p (h w) -> p h w", h=H)

                        def mkinit(vpad=vpad, vimg=vimg, a2d=a2d, ct=ct):
                            nc.vector.memset(vpad[:], 0.0)

                        def mkcopy(vpad=vpad, vimg=vimg):
                            nc.vector.tensor_copy(vpad[:, 1:33, 1:33], vimg)

                        def mkcenter(a2d=a2d, vpad=vpad, ct=ct):
                            nc.vector.tensor_scalar(
                                a2d, vpad[:, 1:33, 1:33], pew[ct][:, 4:5],
                                bpe[ct][:], op0=ALU.mult, op1=ALU.add)

                        ops += [mkinit, mkcopy, mkcenter]
                        taps = [(ky, kx) for ky in range(3) for kx in range(3)
                                if not (ky == 1 and kx == 1)]
                        cur, nxt = a2d, b2d
                        for ky, kx in taps:
                            def mktap(nxt=nxt, cur=cur, vpad=vpad, ky=ky,
                                      kx=kx, ct=ct):
                                nc.vector.scalar_tensor_tensor(
                                    nxt, vpad[:, ky:ky + 32, kx:kx + 32],
                                    pew[ct][:, (3 * ky + kx):(3 * ky + kx) + 1],
                                    cur, op0=ALU.mult, op1=ALU.add)
                            ops.append(mktap)
                            cur, nxt = nxt, cur
                        peacc_all[im][ct] = cur
                    return ops

                def emit_dwconv(im):
                    for op in dwconv_ops(im):
                        op()

                def emit_proj(im):
                    for ct in range(2):
                        nc.vector.tensor_tensor(attn_all[im][ct][:],
                                                attn_all[im][ct][:],
                                                peacc_all[im][ct][:],
                                                op=ALU.add)
                    for ot in range(2):
                        ot_sb = outpool.tile([128, N], F32, tag="osb",
                                             name="ot_sb")
                        for ncol in range(2):
                            pp = ps_small.tile([128, 512], F32, tag="small",
                                               name="pp")
                            for ct in range(2):
                                nc.tensor.matmul(
                                    pp[:], w2t[ct][:, 128 * ot:128 * (ot + 1)],
                                    attn_all[im][ct][:,
                                                     512 * ncol:512 * (ncol + 1)],
                                    start=(ct == 0), stop=(ct == 1))
                            nc.vector.tensor_scalar_add(
                                ot_sb[:, 512 * ncol:512 * (ncol + 1)], pp[:],
                                b2[ot][:])
                        nc.sync.dma_start(
                            out_d[im, 128 * ot:128 * (ot + 1), :], ot_sb[:])

                # ---- minimal img0 prefix: just what group 0 needs ----
                for ot in (0, 2):
                    for nq in range(4):
                        emit_qkv_unit(0, ot, nq)
                for tt in range(3):
                    emit_vt_unit(0, tt)

                # Everything else (rest of img0 preproc, both dwconvs, all of
                # img1 preproc) is injected into the attention pipeline, ONE
                # op per kt slot, ordered by first use, so the in-order
                # DVE/PE never build a burst that stalls ACT.
                def q_unit(im, ot, nq):
                    return lambda: emit_qkv_unit(im, ot, nq)

                def v_unit(im, tt):
                    return lambda: emit_vt_unit(im, tt)

                fillers = []
                fillers += [v_unit(0, tt) for tt in range(3, 8)]
                fillers += [q_unit(0, ot, nq) for ot in (1, 3)
                            for nq in range(4)]
                fillers += [q_unit(0, ot, nq) for ot in (4, 5)
                            for nq in range(4)]
                if B_LOC > 1:
                    fillers += [q_unit(1, ot, nq) for ot in (0, 2)
                                for nq in range(4)]
                    fillers += [v_unit(1, tt) for tt in range(8)]
                fillers += dwconv_ops(0)
                if B_LOC > 1:
                    fillers += [q_unit(1, ot, nq) for ot in (4, 5)
                                for nq in range(4)]
                    fillers += [q_unit(1, ot, nq) for ot in (1, 3)
                                for nq in range(4)]
                    fillers += dwconv_ops(1)

                # ---- attention, software-pipelined across all groups ----
                # 2-head groups g: heads A=2g, B=2g+1 share K/Q tile t=g//2.
                # QK row-tiled (K=16 at 32-aligned rows), one PSUM BANK per
                # head (concurrent row-tiled matmuls need distinct banks).
                # AV col-tiled at psum partitions 0/64 of one bank.
                # QK(kt+1) issues before AV(kt) so the in-order PE never
                # stalls ACT; each group's normalization is deferred into the
                # next group's kt loop.
                seq = [(im, g, qc) for im in range(B_LOC)
                       for g in range(4) for qc in range(2)]
                pqk_store = {}

                def emit_qk(i, kt):
                    im, g, qc = seq[i]
                    t = g // 2
                    jA = (2 * g) % 4
                    pqk = ps_qk.tile([128, 1024], F32, tag="qk", name="pqk")
                    for j, bank in ((jA, 0), (jA + 1, 1)):
                        nc.tensor.matmul(
                            pqk[:, 512 * bank:512 * (bank + 1)],
                            qkv_all[im][t][32 * j:32 * j + 16,
                                           128 * kt:128 * (kt + 1)],
                            qkv_all[im][2 + t][32 * j:32 * j + 16,
                                               512 * qc:512 * (qc + 1)],
                            tile_position=(32 * j, 0))
                    pqk_store[(i, kt)] = pqk

                def emit_norm(i, pav):
                    im, g, qc = seq[i]
                    rec = nrmpool.tile([1, 1024], BF16, tag="rec", name="rec")
                    with nc.allow_low_precision(reason="bf16 softmax den"):
                        nc.vector.reciprocal(rec[:, 0:512], pav[32:33, :])
                        nc.vector.reciprocal(rec[:, 512:1024], pav[96:97, :])
                    pbc = ps_small.tile([128, 512], F32, tag="small",
                                        name="pbc")
                    nc.tensor.matmul(pbc[0:32, :], ones32[:], rec[:, 0:512],
                                     tile_position=(0, 0))
                    nc.tensor.matmul(pbc[64:96, :], ones32[:],
                                     rec[:, 512:1024], tile_position=(0, 64))
                    rb = nrmpool.tile([128, 512], F32, tag="rb", name="rb")
                    nc.vector.tensor_copy(rb[:], pbc[:])
                    ct = g // 2
                    rbase = (g % 2) * 64
                    nc.vector.tensor_tensor(
                        attn_all[im][ct][rbase:rbase + 32,
                                         512 * qc:512 * (qc + 1)],
                        pav[0:32, :], rb[0:32, :], op=ALU.mult)
                    nc.vector.tensor_tensor(
                        attn_all[im][ct][rbase + 32:rbase + 64,
                                         512 * qc:512 * (qc + 1)],
                        pav[64:96, :], rb[64:96, :], op=ALU.mult)

                emit_qk(0, 0)
                norm_pending = []
                for i in range(len(seq)):
                    im, g, qc = seq[i]
                    hA, hB = 2 * g, 2 * g + 1
                    if i == 9:
                        emit_proj(0)
                    pav = ps_av.tile([128, 512], F32, tag="av", name="pav")
                    for kt in range(8):
                        est = exppool.tile([128, 1024], BF16, tag="est",
                                           name="est")
                        nc.scalar.activation(est[:], pqk_store.pop((i, kt))[:],
                                             AF.Exp, scale=SCALE)
                        if kt + 1 < 8:
                            emit_qk(i, kt + 1)
                        elif i + 1 < len(seq):
                            emit_qk(i + 1, 0)
                        nc.tensor.matmul(
                            pav[0:33, :],
                            vts_all[im][kt][:, 33 * hA:33 * hA + 33],
                            est[:, 0:512], start=(kt == 0), stop=(kt == 7),
                            tile_position=(0, 0))
                        nc.tensor.matmul(
                            pav[64:97, :],
                            vts_all[im][kt][:, 33 * hB:33 * hB + 33],
                            est[:, 512:1024], start=(kt == 0), stop=(kt == 7),
                            tile_position=(0, 64))
                        if kt == 2 and norm_pending:
                            emit_norm(*norm_pending.pop(0))
                        elif fillers:
                            fillers.pop(0)()
                    norm_pending.append((i, pav))
                emit_norm(*norm_pending.pop(0))
                if B_LOC > 1:
                    emit_proj(1)
                else:
                    emit_proj(0)
    _split_waits(nc)
    return nc


def kernel(**inputs):
    x = np.asarray(inputs['x'], np.float32)
    Bful, C, Hh, Ww = x.shape
    assert (Bful, C, Hh, Ww) == (B, DIM, H, W)

    key = 'prog1'
    if key not in _cache:
        _cache[key] = _build_program()
    nc = _cache[key]

    wd = _prep_weights(
        np.asarray(inputs['qkv_w'], np.float32), np.asarray(inputs['qkv_gamma'], np.float32),
        np.asarray(inputs['qkv_beta'], np.float32), np.asarray(inputs['qkv_mean'], np.float32),
        np.asarray(inputs['qkv_var'], np.float32),
        np.asarray(inputs['pe_w'], np.float32), np.asarray(inputs['pe_gamma'], np.float32),
        np.asarray(inputs['pe_beta'], np.float32), np.asarray(inputs['pe_mean'], np.float32),
        np.asarray(inputs['pe_var'], np.float32),
        np.asarray(inputs['proj_w'], np.float32), np.asarray(inputs['proj_gamma'], np.float32),
        np.asarray(inputs['proj_beta'], np.float32), np.asarray(inputs['proj_mean'], np.float32),
        np.asarray(inputs['proj_var'], np.float32),
    )

    xr = x.reshape(B, 256, N).astype(ml_dtypes.bfloat16)
    in_maps = []
    for c in range(N_CORES):
        m = dict(wd)
        m['x'] = xr[B_LOC * c:B_LOC * (c + 1)]
        in_maps.append(m)

    res = run_bass_kernel_spmd(nc, in_maps, list(range(N_CORES)))
    out = np.concatenate([r['out'] for r in res.results], axis=0)
    return out.reshape(B, 256, H, W).astype(np.float32)


def make_runner(**inputs):
    """Build (jitted_fn, concat_inputs, zero_outs, postprocess) for benchmarking.

    Mirrors bass2jax.run_bass_via_pjrt's multi-core path but without donation
    so the same buffers can be re-executed for timing."""
    import jax
    from jax.sharding import Mesh, PartitionSpec
    from jax.experimental.shard_map import shard_map
    from concourse import bass2jax, mybir as _mb

    x = np.asarray(inputs['x'], np.float32)
    wd = _prep_weights(**{k: np.asarray(inputs[k], np.float32) for k in (
        'qkv_w', 'qkv_gamma', 'qkv_beta', 'qkv_mean', 'qkv_var',
        'pe_w', 'pe_gamma', 'pe_beta', 'pe_mean', 'pe_var',
        'proj_w', 'proj_gamma', 'proj_beta', 'proj_mean', 'proj_var')})
    reps = int(inputs.get('_bench_reps', 1))
    ck = f'prog{reps}'
    if ck not in _cache:
        _cache[ck] = _build_program(reps)
    nc = _cache[ck]
    xr = x.reshape(B, 256, N).astype(ml_dtypes.bfloat16)
    in_maps = []
    for c in range(N_CORES):
        m = dict(wd)
        m['x'] = xr[B_LOC * c:B_LOC * (c + 1)]
        in_maps.append(m)

    bass2jax.install_neuronx_cc_hook()
    in_names, out_names, out_avals, zero_outs = [], [], [], []
    for alloc in nc.m.functions[0].allocations:
        if not isinstance(alloc, _mb.MemoryLocationSet):
            continue
        name = alloc.memorylocations[0].name
        if alloc.kind == "ExternalInput":
            if nc.partition_id_tensor and name == nc.partition_id_tensor.name:
                continue
            in_names.append(name)
        elif alloc.kind == "ExternalOutput":
            out_names.append(name)
            out_avals.append(jax.core.ShapedArray(
                tuple(alloc.tensor_shape), _mb.dt.np(alloc.dtype)))
            zero_outs.append(np.zeros(tuple(alloc.tensor_shape),
                                      _mb.dt.np(alloc.dtype)))
    n_params = len(in_names)
    all_names = in_names + out_names

    pname = nc.partition_id_tensor.name if nc.partition_id_tensor else None

    def _body(*args):
        operands = list(args)
        names = list(all_names)
        if pname is not None:
            operands.append(bass2jax.partition_id_tensor())
            names.append(pname)
        outs = bass2jax._bass_exec_p.bind(
            *operands,
            out_avals=tuple(out_avals),
            in_names=tuple(names),
            out_names=tuple(out_names),
            lowering_input_output_aliases=(),
            sim_require_finite=True,
            sim_require_nnan=True,
            nc=nc,
        )
        return tuple(outs)

    devices = jax.devices()[:N_CORES]
    mesh = Mesh(np.asarray(devices), ("core",))
    nin = n_params + len(out_names)
    sharded = jax.jit(
        shard_map(_body, mesh=mesh,
                  in_specs=(PartitionSpec("core"),) * nin,
                  out_specs=(PartitionSpec("core"),) * len(out_names),
                  check_rep=False),
        keep_unused=True,
    )
    per_core = [[np.asarray(m[nm]) for nm in in_names] for m in in_maps]
    concat_in = [np.concatenate([per_core[c][i] for c in range(N_CORES)], axis=0)
                 for i in range(n_params)]
    concat_zeros = [np.zeros((N_CORES * z.shape[0], *z.shape[1:]), z.dtype)
                    for z in zero_outs]

    def post(out_arrs):
        full = np.asarray(out_arrs[0]).reshape(N_CORES, B_LOC, 256, N)
        return full.reshape(B, 256, N).reshape(B, 256, H, W).astype(np.float32)

    return sharded, concat_in, concat_zeros, post



# revision 43
# speedup vs baseline: 1.5158x; 1.5158x over previous
"""TRN2 Bass kernel for nn_Attention_59081570125142.

MobileViT-style attention block:
  qkv = BN(1x1conv(x)); per-head attention over N=1024 tokens
  (key_dim=16, head_dim=32, 8 heads); pos_enc = BN(dwconv3x3(v));
  out = BN(1x1conv(v_attn + pos_enc)).

Sharding: data-parallel over batch B=16 across 8 cores (2 images/core).
All BN folded into conv weights/biases on host. Matmuls in bf16,
accumulation fp32 in PSUM.

Softmax layout: logits computed KEY-major (logitsT[k, q] via lhsT=k_head,
rhs=q_head) so the softmax normalizer rides as a ones-column in the AV
matmul. The AV matmul is FLIPPED: est chunks [128k, 128q] are the
stationary lhsT and vts [128k, 33] streams as rhs, so the AV output land
QUERY-major ([q, head-dims]) and the denominator is a per-partition
column -> normalization is one strided reciprocal [128,8] plus one
broadcast multiply per group instead of per-column broadcast matmuls.
A cheap PE transpose (16x [128,128]) restores channel-major, with the
pos_enc add fused into the PSUM evacuation op.
QK packs 4 heads in the PE array via row tiling (K=16 each).
"""
import sys

sys.path.insert(0, '/opt/trn_rl_repo')

import numpy as np
import ml_dtypes

import concourse.bass as bass
import concourse.mybir as mybir
from concourse import tile
from concourse.bass_utils import run_bass_kernel_spmd

F32 = mybir.dt.float32
F32R = mybir.dt.float32r
BF16 = mybir.dt.bfloat16
AF = mybir.ActivationFunctionType
ALU = mybir.AluOpType

N_CORES = 8
B = 16
B_LOC = B // N_CORES          # 2 images per core
DIM = 256
NUM_HEADS = 8
HEAD_DIM = 32
KEY_DIM = 16
QKV_OUT = 512
N = 1024                      # 32*32 tokens
H = W = 32
SCALE = KEY_DIM ** -0.5       # 0.25
BN_EPS = 1e-3

_cache = {}


def _fold_bn(w, gamma, beta, mean, var):
    inv = gamma / np.sqrt(var + BN_EPS)
    return w * inv[:, None], beta - mean * inv


def _prep_weights(qkv_w, qkv_gamma, qkv_beta, qkv_mean, qkv_var,
                  pe_w, pe_gamma, pe_beta, pe_mean, pe_var,
                  proj_w, proj_gamma, proj_beta, proj_mean, proj_var):
    """Host-side BN folding + layout rearrangement."""
    W1, b1 = _fold_bn(qkv_w[:, :, 0, 0].astype(np.float32), qkv_gamma, qkv_beta,
                      qkv_mean, qkv_var)                      # (512, 256), (512,)
    W2, b2 = _fold_bn(proj_w[:, :, 0, 0].astype(np.float32), proj_gamma, proj_beta,
                      proj_mean, proj_var)                    # (256, 256), (256,)
    invpe = pe_gamma / np.sqrt(pe_var + BN_EPS)
    PW = (pe_w[:, 0] * invpe[:, None, None]).astype(np.float32)  # (256, 3, 3)
    bpe = (pe_beta - pe_mean * invpe).astype(np.float32)         # (256,)

    # channel-major qkv pass: out = W1T_cm.T @ x, 6 output tiles of 128:
    #  t0/t1: K heads 0-3 / 4-7 at 32-aligned slots (16 rows used per head)
    #  t2/t3: Q likewise;  t4/t5: V natural order (head h -> rows 32h..32h+32)
    w1t_cm = np.zeros((256, 768), np.float32)
    b1_cm = np.zeros((768,), np.float32)
    for h in range(NUM_HEADS):
        t = h // 4
        j = h % 4
        # K
        cols = t * 128 + 32 * j + np.arange(16)
        chans = h * 64 + 16 + np.arange(16)
        w1t_cm[:, cols] = W1[chans].T
        b1_cm[cols] = b1[chans]
        # Q
        cols = 256 + t * 128 + 32 * j + np.arange(16)
        chans = h * 64 + np.arange(16)
        w1t_cm[:, cols] = W1[chans].T
        b1_cm[cols] = b1[chans]
        # V
        cols = 512 + h * 32 + np.arange(32)
        chans = h * 64 + 32 + np.arange(32)
        w1t_cm[:, cols] = W1[chans].T
        b1_cm[cols] = b1[chans]

    # token-major v pass: vT[tok, 33h+d] = sum_c x[c,tok] * W1'[h*64+32+d, c];
    # col 33h+32 gets 0 from the matmul and 1.0 from the bias tile (ones col
    # for the fused softmax denominator).
    wvt = np.zeros((256, 264), np.float32)
    bvb = np.zeros((264,), np.float32)
    for h in range(NUM_HEADS):
        cols = 33 * h + np.arange(32)
        chans = h * 64 + 32 + np.arange(32)
        wvt[:, cols] = W1[chans].T
        bvb[cols] = b1[chans]
        bvb[33 * h + 32] = 1.0

    bf = ml_dtypes.bfloat16
    return dict(
        w1t=w1t_cm.astype(bf),                  # (256, 768)
        b1=b1_cm.reshape(768, 1).astype(np.float32),
        wvt=wvt.astype(bf),                     # (256, 264)
        bvb=np.broadcast_to(bvb, (128, 264)).copy().astype(np.float32),
        w2t=W2.T.copy().astype(bf),             # (256, 256)
        b2=b2.reshape(256, 1).astype(np.float32),
        pew=PW.reshape(256, 9).astype(np.float32),
        bpe=bpe.reshape(256, 1).astype(np.float32),
        ident=np.eye(128, dtype=bf),            # (128, 128) transpose identity
    )


def _split_waits(nc, cap=1, caps=None):
    """Walrus rejects instructions with more than ~1-2 semaphore waits
    ("Too many sync wait commands"). Hoist excess waits onto same-engine
    EventSemaphore carrier instructions inserted just before the offender —
    the engine stalls at the carriers first, semantically identical."""
    n = 0
    caps = caps or {}
    for f in nc.m.functions:
        for blk in f.blocks:
            insts = blk.instructions
            out = []
            for inst in insts:
                si = inst.sync_info
                waits = list(si.on_wait) if si else []
                # ACT executes strictly in order, so a wait on its OWN engine
                # semaphore (WAW guard for est-slot reuse) is redundant — but
                # it forces each exp to eat the previous one's write-ack
                # latency. Drop those.
                if (str(inst.engine) == "EngineType.Activation"
                        and waits and inst.opcode == "Activation"):
                    kept = [w for w in waits
                            if not str(w.ant_name).startswith("Activation")]
                    if len(kept) != len(waits):
                        waits = kept
                        si = mybir.SyncInfo(on_wait=waits,
                                            on_update=list(si.on_update))
                        inst.sync_info = si
                icap = caps.get(str(inst.opcode), cap)
                if len(waits) > icap and inst.opcode != "EventSemaphore":
                    extra, keep = waits[:-icap], waits[-icap:]
                    for k, w in enumerate(extra):
                        ev = mybir.InstEventSemaphore(
                            name=f"{inst.name}-sw{k}", ins=[], outs=[],
                            sync_info=mybir.SyncInfo(on_wait=[w], on_update=[]),
                        )
                        ev.engine = inst.engine
                        out.append(ev)
                        n += 1
                    inst.sync_info = mybir.SyncInfo(
                        on_wait=keep, on_update=list(si.on_update))
                out.append(inst)
            blk.instructions = out
    return n


def _build_program(reps=1):
    nc = bass.Bass()
    x_d = nc.declare_dram_parameter("x", [B_LOC, 256, N], BF16, isOutput=False)
    w1t_d = nc.declare_dram_parameter("w1t", [256, 768], BF16, isOutput=False)
    b1_d = nc.declare_dram_parameter("b1", [768, 1], F32, isOutput=False)
    wvt_d = nc.declare_dram_parameter("wvt", [256, 264], BF16, isOutput=False)
    bvb_d = nc.declare_dram_parameter("bvb", [128, 264], F32, isOutput=False)
    w2t_d = nc.declare_dram_parameter("w2t", [256, 256], BF16, isOutput=False)
    b2_d = nc.declare_dram_parameter("b2", [256, 1], F32, isOutput=False)
    pew_d = nc.declare_dram_parameter("pew", [256, 9], F32, isOutput=False)
    bpe_d = nc.declare_dram_parameter("bpe", [256, 1], F32, isOutput=False)
    ident_d = nc.declare_dram_parameter("ident", [128, 128], BF16, isOutput=False)
    out_d = nc.declare_dram_parameter("out", [B_LOC, 256, N], F32, isOutput=True)

    with tile.TileContext(nc) as tc:
        with (
            tc.tile_pool(name="const", bufs=1) as cpool,
            tc.tile_pool(name="xp", bufs=1) as xpool,
            tc.tile_pool(name="qkv", bufs=1) as qkvpool,
            tc.tile_pool(name="vt", bufs=1) as vtpool,
            tc.tile_pool(name="pe", bufs=1) as pepool,
            tc.tile_pool(name="petmp", bufs=3) as petmp,
            tc.tile_pool(name="exp", bufs=6) as exppool,
            tc.tile_pool(name="attn", bufs=1) as attnpool,
            tc.tile_pool(name="attnT", bufs=1) as attntpool,
            tc.tile_pool(name="nrm", bufs=3) as nrmpool,
            tc.tile_pool(name="outp", bufs=4) as outpool,
            tc.tile_pool(name="ps_small", bufs=1, space="PSUM") as ps_small, # 1 bank (qkv/vt/proj/transpose)
            tc.tile_pool(name="ps_qk", bufs=3, space="PSUM") as ps_qk,       # 6 banks
            tc.tile_pool(name="ps_av", bufs=1, space="PSUM") as ps_av,       # 1 bank
        ):
            # ---- load constants ----
            w1t = [cpool.tile([128, 768], BF16, tag=f"w1t{i}", name=f"w1t{i}") for i in range(2)]
            wvt = [cpool.tile([128, 264], BF16, tag=f"wvt{i}", name=f"wvt{i}") for i in range(2)]
            w2t = [cpool.tile([128, 256], BF16, tag=f"w2t{i}", name=f"w2t{i}") for i in range(2)]
            b2 = [cpool.tile([128, 1], F32, tag=f"b2{i}", name=f"b2{i}") for i in range(2)]
            pew = [cpool.tile([128, 9], F32, tag=f"pew{i}", name=f"pew{i}") for i in range(2)]
            bpe = [cpool.tile([128, 1], F32, tag=f"bpe{i}", name=f"bpe{i}") for i in range(2)]
            b1 = [cpool.tile([128, 1], F32, tag=f"b1{i}", name=f"b1{i}") for i in range(6)]
            bvb = cpool.tile([128, 264], F32, tag="bvb")
            ident = cpool.tile([128, 128], BF16, tag="ident", name="ident")
            xs = [[xpool.tile([128, N], BF16, tag=f"x{im}{ct}", name=f"x{im}{ct}") for ct in range(2)]
                  for im in range(B_LOC)]
            # DMA issue cost is ~0.5-0.8us each and serial per queue: put the
            # startup critical path (x img0, w1t, K/Q biases, vt weights)
            # first and split it across the two HWDGE queues (SP + ACT) so
            # the first QK->exp fires ~5us in, not ~19us.
            nc.sync.dma_start(xs[0][0][:], x_d[0, 0:128, :])
            nc.scalar.dma_start(xs[0][1][:], x_d[0, 128:256, :])
            nc.sync.dma_start(w1t[0][:], w1t_d[0:128, :])
            nc.scalar.dma_start(w1t[1][:], w1t_d[128:256, :])
            nc.sync.dma_start(b1[0][:], b1_d[0:128, :])
            nc.scalar.dma_start(b1[2][:], b1_d[256:384, :])
            nc.sync.dma_start(wvt[0][:], wvt_d[0:128, :])
            nc.scalar.dma_start(wvt[1][:], wvt_d[128:256, :])
            nc.sync.dma_start(bvb[:], bvb_d[:])
            for ot in (1, 3, 4, 5):
                nc.sync.dma_start(b1[ot][:], b1_d[128 * ot:128 * (ot + 1), :])
            for ct in range(2):
                nc.sync.dma_start(xs[1][ct][:], x_d[1, 128 * ct:128 * (ct + 1), :])
            for ct in range(2):
                nc.sync.dma_start(pew[ct][:], pew_d[128 * ct:128 * (ct + 1), :])
                nc.sync.dma_start(bpe[ct][:], bpe_d[128 * ct:128 * (ct + 1), :])
            nc.sync.dma_start(ident[:], ident_d[:])
            for ct in range(2):
                nc.sync.dma_start(w2t[ct][:], w2t_d[128 * ct:128 * (ct + 1), :])
                nc.sync.dma_start(b2[ct][:], b2_d[128 * ct:128 * (ct + 1), :])

            for _rep in range(reps):
                qkv_all = [[qkvpool.tile([128, N], BF16, tag=f"qkv{im}{ot}",
                                         name=f"qkv{im}{ot}") for ot in range(6)]
                           for im in range(B_LOC)]
                vts_all = [[vtpool.tile([128, 264], BF16, tag=f"vt{im}{tt}",
                                        name=f"vt{im}{tt}") for tt in range(8)]
                           for im in range(B_LOC)]
                attn_cm = [[attnpool.tile([128, N], BF16, tag=f"attn{im}{ct}",
                                          name=f"attn{im}{ct}")
                            for ct in range(2)] for im in range(B_LOC)]
                # query-major normalized attention, [q, 8 qchunks x 256 chans]
                attnT = [attntpool.tile([128, 2048], BF16, tag=f"attnT{im}",
                                        name=f"attnT{im}") for im in range(B_LOC)]
                peacc_all = [[None, None] for _ in range(B_LOC)]

                def emit_qkv_unit(im, ot, nq, alt=False):
                    # N=256 chunks keep per-filler-slot PE load under the
                    # ACT period so injected preproc never stalls the pipeline.
                    # alt=True borrows the (pre-attention) av bank so prefix
                    # units run as two parallel chains instead of one.
                    pool, tag = (ps_av, "av") if alt else (ps_small, "small")
                    pq = pool.tile([128, 256], F32, tag=tag, name="pq")
                    for ct in range(2):
                        nc.tensor.matmul(
                            pq[:], w1t[ct][:, 128 * ot:128 * (ot + 1)],
                            xs[im][ct][:, 256 * nq:256 * (nq + 1)],
                            start=(ct == 0), stop=(ct == 1))
                    nc.vector.tensor_scalar_add(
                        qkv_all[im][ot][:, 256 * nq:256 * (nq + 1)],
                        pq[:], b1[ot][:])

                def emit_vt_unit(im, tt):
                    pv = ps_small.tile([128, 512], F32, tag="small", name="pv")
                    for ct in range(2):
                        nc.tensor.matmul(
                            pv[:, 0:264],
                            xs[im][ct][:, 128 * tt:128 * (tt + 1)], wvt[ct][:],
                            start=(ct == 0), stop=(ct == 1))
                    nc.vector.tensor_tensor(vts_all[im][tt][:], pv[:, 0:264],
                                            bvb[:], op=ALU.add)

                def dwconv_ops(im):
                    # depthwise 3x3 on v (bf16), SAME-pad via interior sub-
                    # views (out-of-range contributions simply absent).
                    # Center tap: tensor_scalar (4x DVE mode) initializes acc
                    # incl. bias. Taps 1-4 on DVE as premult (tensor_scalar,
                    # 4x) + in-place shifted add (tensor_tensor, 2x). Taps
                    # 5-8 on the otherwise-idle GPSIMD engine (in-place
                    # scalar_tensor_tensor), emitted with the last DVE tap so
                    # they claim no filler slots.
                    ops = []
                    for ct in range(2):
                        acc = pepool.tile([128, N], BF16,
                                          tag=f"peacc{im}{ct}",
                                          name=f"peacc{im}{ct}")
                        a2d = acc[:].rearrange("p (h w) -> p h w", h=H)
                        vimg = qkv_all[im][4 + ct][:].rearrange(
                            "p (h w) -> p h w", h=H)

                        def mkcenter(a2d=a2d, vimg=vimg, ct=ct):
                            nc.vector.tensor_scalar(
                                a2d, vimg, pew[ct][:, 4:5],
                                bpe[ct][:], op0=ALU.mult, op1=ALU.add)

                        ops.append(mkcenter)
                        taps = [(ky, kx) for ky in range(3) for kx in range(3)
                                if not (ky == 1 and kx == 1)]

                        def geom(ky, kx):
                            oy0, oy1 = max(0, 1 - ky), 32 - max(0, ky - 1)
                            ox0, ox1 = max(0, 1 - kx), 32 - max(0, kx - 1)
                            return (oy0, oy1, ox0, ox1,
                                    oy0 + ky - 1, oy1 + ky - 1,
                                    ox0 + kx - 1, ox1 + kx - 1)

                        def dve_tap(ky, kx, ct=ct, a2d=a2d, vimg=vimg):
                            oy0, oy1, ox0, ox1, iy0, iy1, ix0, ix1 = \
                                geom(ky, kx)
                            tmp = petmp.tile([128, N], BF16, tag="dwtmp",
                                             name="dwtmp")
                            t2d = tmp[:].rearrange("p (h w) -> p h w", h=H)
                            nc.vector.tensor_scalar(
                                tmp[:], qkv_all[im][4 + ct][:],
                                pew[ct][:, (3 * ky + kx):(3 * ky + kx) + 1],
                                None, op0=ALU.mult)
                            nc.vector.tensor_tensor(
                                a2d[:, oy0:oy1, ox0:ox1],
                                a2d[:, oy0:oy1, ox0:ox1],
                                t2d[:, iy0:iy1, ix0:ix1], op=ALU.add)

                        def pool_tap(ky, kx, ct=ct, a2d=a2d, vimg=vimg):
                            # GPSIMD's ISA has no scalar_tensor_tensor: DVE
                            # premultiplies (tensor_scalar, 4x mode) and the
                            # Pool engine does the in-place shifted add
                            oy0, oy1, ox0, ox1, iy0, iy1, ix0, ix1 = \
                                geom(ky, kx)
                            tmp = petmp.tile([128, N], BF16, tag="pooltmp",
                                             name="pooltmp", bufs=4)
                            t2d = tmp[:].rearrange("p (h w) -> p h w", h=H)
                            nc.vector.tensor_scalar(
                                tmp[:], qkv_all[im][4 + ct][:],
                                pew[ct][:, (3 * ky + kx):(3 * ky + kx) + 1],
                                None, op0=ALU.mult)
                            nc.gpsimd.tensor_tensor(
                                a2d[:, oy0:oy1, ox0:ox1],
                                a2d[:, oy0:oy1, ox0:ox1],
                                t2d[:, iy0:iy1, ix0:ix1], op=ALU.add)

                        for k in range(4):
                            ops.append(lambda t=taps[k], f=dve_tap: f(*t))
                        for k in range(4, 8):
                            ops.append(lambda t=taps[k], g=pool_tap: g(*t))
                        peacc_all[im][ct] = acc
                    return ops

                ots_all = [{} for _ in range(B_LOC)]

                def post_half(im, h):
                    # one qc-half (512 queries): transposes back to channel-
                    # major + fused pos_enc add + proj + out DMA, as
                    # single-op closures for filler slots
                    ops = []
                    ptrs = {}

                    def mktr(ct, part):
                        def f():
                            if part == 0:
                                ptrs[ct] = ps_small.tile(
                                    [128, 512], BF16, tag="small", name="ptr")
                            for a in (4 * h + 2 * part, 4 * h + 2 * part + 1):
                                nc.tensor.transpose(
                                    ptrs[ct][:, 128 * (a - 4 * h):
                                             128 * (a - 4 * h) + 128],
                                    attnT[im][:, 256 * a + 128 * ct:
                                              256 * a + 128 * ct + 128],
                                    ident[:])
                        return f

                    def mkevac(ct):
                        def f():
                            nc.vector.tensor_tensor(
                                attn_cm[im][ct][:, 512 * h:512 * h + 512],
                                ptrs.pop(ct)[:],
                                peacc_all[im][ct][:, 512 * h:512 * h + 512],
                                op=ALU.add)
                        return f

                    def mkproj(ot):
                        def f():
                            ots = ots_all[im]
                            if ot not in ots:
                                ots[ot] = outpool.tile([128, N], F32,
                                                       tag="osb", name="ot_sb")
                            pp = ps_small.tile([128, 512], F32, tag="small",
                                               name="pp")
                            for ct in range(2):
                                nc.tensor.matmul(
                                    pp[:], w2t[ct][:, 128 * ot:128 * (ot + 1)],
                                    attn_cm[im][ct][:, 512 * h:512 * h + 512],
                                    start=(ct == 0), stop=(ct == 1))
                            nc.vector.tensor_scalar_add(
                                ots[ot][:, 512 * h:512 * h + 512], pp[:],
                                b2[ot][:])
                            nc.sync.dma_start(
                                out_d[im, 128 * ot:128 * (ot + 1),
                                      512 * h:512 * h + 512],
                                ots[ot][:, 512 * h:512 * h + 512])
                        return f

                    ops += [mktr(0, 0), mktr(0, 1), mkevac(0),
                            mktr(1, 0), mktr(1, 1), mkevac(1)]
                    ops += [mkproj(0), mkproj(1)]
                    return ops

                # ---- minimal img0 prefix: what (qc0, g0/g1) needs first.
                # Units alternate between the small/av banks (two parallel
                # chains); the first QK is emitted as soon as its three
                # producer units are, so the PE reaches it early.

                # Everything else (rest of img0 preproc, both dwconvs, all of
                # img1 preproc) is injected into the attention pipeline, ONE
                # closure per kt slot, ordered so each unit's EMISSION
                # precedes its first reader's emission (the dep tracker only
                # sees program order).
                def q_unit(im, ot, nq):
                    return lambda: emit_qkv_unit(im, ot, nq)

                def v_unit(im, tt):
                    return lambda: emit_vt_unit(im, tt)

                fillers = []
                fillers += [v_unit(0, 2), v_unit(0, 3), q_unit(0, 0, 3),
                            v_unit(0, 4), v_unit(0, 5), v_unit(0, 6),
                            v_unit(0, 7)]
                fillers += [q_unit(0, 1, 0), q_unit(0, 3, 0), q_unit(0, 3, 1),
                            q_unit(0, 1, 1), q_unit(0, 1, 2), q_unit(0, 1, 3)]
                fillers += [q_unit(0, 2, 2), q_unit(0, 2, 3),
                            q_unit(0, 3, 2), q_unit(0, 3, 3)]
                fillers += [q_unit(0, ot, nq) for ot in (4, 5)
                            for nq in range(4)]
                if B_LOC > 1:
                    # im1's boundary-critical units go ahead of im0's dwconv:
                    # the shared psum bank serializes preproc units, so they
                    # need a long runway before i==8 consumes them
                    fillers += [q_unit(1, 0, 0), q_unit(1, 2, 0),
                                q_unit(1, 2, 1), v_unit(1, 0),
                                q_unit(1, 0, 1), v_unit(1, 1),
                                v_unit(1, 2), v_unit(1, 3)]
                fillers += dwconv_ops(0)
                if B_LOC > 1:
                    fillers += [v_unit(1, tt) for tt in range(4, 8)]
                    fillers += [q_unit(1, 0, 2), q_unit(1, 0, 3),
                                q_unit(1, 2, 2), q_unit(1, 2, 3)]
                    fillers += [q_unit(1, 1, 0), q_unit(1, 1, 1),
                                q_unit(1, 3, 0), q_unit(1, 3, 1)]
                    fillers += [q_unit(1, 1, 2), q_unit(1, 1, 3),
                                q_unit(1, 3, 2), q_unit(1, 3, 3)]
                    fillers += [q_unit(1, ot, nq) for ot in (4, 5)
                                for nq in range(4)]
                    fillers += dwconv_ops(1)

                # ---- attention, software-pipelined across all groups ----
                # 2-head groups g: heads A=2g, B=2g+1 share K/Q tile t=g//2.
                # QK row-tiled (K=16 at 32-aligned rows), one PSUM BANK per
                # head. AV is flipped (est stationary, vts moving): out
                # [128 queries, (4 qchunks x 2 heads x 33)] in one bank.
                # QK(kt+1) issues before AV(kt) so the in-order PE never
                # stalls ACT; each group's normalization is deferred into the
                # next group's kt loop.
                # qc-major per image: all groups' qc0 first, so the first
                # half's post work (transpose+proj) can overlap the second
                # half's attention
                seq = [(im, g, qc) for im in range(B_LOC)
                       for qc in range(2) for g in range(4)]
                pqk_store = {}

                def emit_qk(i, kt):
                    im, g, qc = seq[i]
                    t = g // 2
                    jA = (2 * g) % 4
                    pqk = ps_qk.tile([128, 1024], F32, tag="qk", name="pqk")
                    for j, bank in ((jA, 0), (jA + 1, 1)):
                        nc.tensor.matmul(
                            pqk[:, 512 * bank:512 * (bank + 1)],
                            qkv_all[im][t][32 * j:32 * j + 16,
                                           128 * kt:128 * (kt + 1)],
                            qkv_all[im][2 + t][32 * j:32 * j + 16,
                                               512 * qc:512 * (qc + 1)],
                            tile_position=(32 * j, 0))
                    pqk_store[(i, kt)] = pqk

                def emit_norm(i, pav):
                    im, g, qc = seq[i]
                    rz = nrmpool.tile([128, 8], BF16, tag="rz", name="rz")
                    pav3 = pav.rearrange("p (c hp d) -> p c hp d", c=4, d=33)
                    rz3 = rz[:].rearrange("p (c hp) -> p c hp", c=4)
                    with nc.allow_low_precision(reason="bf16 softmax den"):
                        nc.vector.reciprocal(rz3, pav3[:, :, :, 32])
                    outap = attnT[im][:].rearrange(
                        "p (a g hp d) -> p a g hp d", g=4, hp=2, d=32)[
                        :, 4 * qc:4 * qc + 4, g]
                    rzb = rz3.unsqueeze(3).broadcast_to([128, 4, 2, 32])
                    nc.vector.tensor_tensor(outap, pav3[:, :, :, 0:32], rzb,
                                            op=ALU.mult)

                # Schraudolph fast-exp on DVE for ~1/5 of the tiles to
                # offload the bottleneck ACT engine: est = bitcast<bf16>(
                # int16(round(logit * SCALE*128/ln2 + 16256))). ~3% per-
                # element error that largely cancels in the softmax ratio.
                SCHR_A = float(SCALE * 128.0 / np.log(2.0))
                SCHR_B = 16256.0

                def schr(i, kt):
                    # the DVE est op gates QK(kt+3) via the psum rotation, so
                    # keep it off the i-ranges where the DVE queue is flooded
                    # by dwconv fillers (i 4-5, 11-12) and the im1 preproc
                    # catch-up (i 8)
                    if i in (4, 5, 8, 11, 12):
                        return False
                    if i == 15:
                        return kt % 2 == 1
                    return (8 * i + kt) % 3 == 2

                emit_qkv_unit(0, 0, 0)             # K nq0
                emit_qkv_unit(0, 2, 0, alt=True)   # Q nq0
                emit_qkv_unit(0, 2, 1)             # Q nq1
                emit_qk(0, 0)
                emit_qkv_unit(0, 0, 1, alt=True)   # K nq1
                emit_vt_unit(0, 0)
                emit_qkv_unit(0, 0, 2, alt=True)   # K nq2
                emit_vt_unit(0, 1)
                norm_pending = []
                post_fillers = []
                for i in range(len(seq)):
                    im, g, qc = seq[i]
                    hA, hB = 2 * g, 2 * g + 1
                    if norm_pending:
                        # norm(i-1) before pav(i): with ps_av bufs=1 the
                        # AV(i) matmuls reuse the bank norm(i-1) reads
                        emit_norm(*norm_pending.pop(0))
                    if i in (6, 9, 14):
                        # previous qc-half fully normalized; queue its
                        # transposes+proj ahead of other fillers (h0 posts
                        # wait an extra group so the dwconv tail is emitted
                        # before the evac that reads peacc)
                        pim, ph = {6: (0, 0), 9: (0, 1), 14: (1, 0)}[i]
                        post_fillers += post_half(pim, ph)
                    pav = ps_av.tile([128, 264], F32, tag="av", name="pav")
                    for kt in range(8):
                        est = exppool.tile([128, 1024], BF16, tag="est",
                                           name="est")
                        pqk = pqk_store.pop((i, kt))
                        if schr(i, kt):
                            nc.vector.tensor_scalar(
                                est[:].bitcast(mybir.dt.int16), pqk[:],
                                SCHR_A, SCHR_B, op0=ALU.mult, op1=ALU.add)
                        else:
                            nc.scalar.activation(est[:], pqk[:],
                                                 AF.Exp, scale=SCALE)
                        if kt + 1 < 8:
                            emit_qk(i, kt + 1)
                        elif i + 1 < len(seq):
                            emit_qk(i + 1, 0)
                        # start=True only on the FIRST matmul of the bank:
                        # on HW (and in CoreSim) start marks the whole 2KB
                        # zero-region pending-zero, so later start=False
                        # writes overwrite-where-pending and accumulate after.
                        for c in range(4):
                            for hp in range(2):
                                nc.tensor.matmul(
                                    pav[:, 66 * c + 33 * hp:
                                        66 * c + 33 * hp + 33],
                                    est[:, 512 * hp + 128 * c:
                                        512 * hp + 128 * c + 128],
                                    vts_all[im][kt][:, 33 * (2 * g + hp):
                                                    33 * (2 * g + hp) + 33],
                                    start=(kt == 0 and c == 0 and hp == 0),
                                    stop=(kt == 7),
                                    skip_group_check=True)
                        if post_fillers and (kt > 2 or i not in (6, 9, 14)):
                            post_fillers.pop(0)()
                        elif fillers:
                            fillers.pop(0)()
                    norm_pending.append((i, pav))
                emit_norm(*norm_pending.pop(0))
                for op in fillers:
                    op()
                for op in post_fillers:
                    op()
                tail = (1, 1) if B_LOC > 1 else (0, 1)
                for op in post_half(*tail):
                    op()
    _split_waits(nc)
    return nc


def kernel(**inputs):
    x = np.asarray(inputs['x'], np.float32)
    Bful, C, Hh, Ww = x.shape
    assert (Bful, C, Hh, Ww) == (B, DIM, H, W)

    key = 'prog1'
    if key not in _cache:
        _cache[key] = _build_program()
    nc = _cache[key]

    wd = _prep_weights(
        np.asarray(inputs['qkv_w'], np.float32), np.asarray(inputs['qkv_gamma'], np.float32),
        np.asarray(inputs['qkv_beta'], np.float32), np.asarray(inputs['qkv_mean'], np.float32),
        np.asarray(inputs['qkv_var'], np.float32),
        np.asarray(inputs['pe_w'], np.float32), np.asarray(inputs['pe_gamma'], np.float32),
        np.asarray(inputs['pe_beta'], np.float32), np.asarray(inputs['pe_mean'], np.float32),
        np.asarray(inputs['pe_var'], np.float32),
        np.asarray(inputs['proj_w'], np.float32), np.asarray(inputs['proj_gamma'], np.float32),
        np.asarray(inputs['proj_beta'], np.float32), np.asarray(inputs['proj_mean'], np.float32),
        np.asarray(inputs['proj_var'], np.float32),
    )

    xr = x.reshape(B, 256, N).astype(ml_dtypes.bfloat16)
    in_maps = []
    for c in range(N_CORES):
        m = dict(wd)
        m['x'] = xr[B_LOC * c:B_LOC * (c + 1)]
        in_maps.append(m)

    res = run_bass_kernel_spmd(nc, in_maps, list(range(N_CORES)))
    out = np.concatenate([r['out'] for r in res.results], axis=0)
    return out.reshape(B, 256, H, W).astype(np.float32)


def make_runner(**inputs):
    """Build (jitted_fn, concat_inputs, zero_outs, postprocess) for benchmarking.

    Mirrors bass2jax.run_bass_via_pjrt's multi-core path but without donation
    so the same buffers can be re-executed for timing."""
    import jax
    from jax.sharding import Mesh, PartitionSpec
    from jax.experimental.shard_map import shard_map
    from concourse import bass2jax, mybir as _mb

    x = np.asarray(inputs['x'], np.float32)
    wd = _prep_weights(**{k: np.asarray(inputs[k], np.float32) for k in (
        'qkv_w', 'qkv_gamma', 'qkv_beta', 'qkv_mean', 'qkv_var',
        'pe_w', 'pe_gamma', 'pe_beta', 'pe_mean', 'pe_var',
        'proj_w', 'proj_gamma', 'proj_beta', 'proj_mean', 'proj_var')})
    reps = int(inputs.get('_bench_reps', 1))
    ck = f'prog{reps}'
    if ck not in _cache:
        _cache[ck] = _build_program(reps)
    nc = _cache[ck]
    xr = x.reshape(B, 256, N).astype(ml_dtypes.bfloat16)
    in_maps = []
    for c in range(N_CORES):
        m = dict(wd)
        m['x'] = xr[B_LOC * c:B_LOC * (c + 1)]
        in_maps.append(m)

    bass2jax.install_neuronx_cc_hook()
    in_names, out_names, out_avals, zero_outs = [], [], [], []
    for alloc in nc.m.functions[0].allocations:
        if not isinstance(alloc, _mb.MemoryLocationSet):
            continue
        name = alloc.memorylocations[0].name
        if alloc.kind == "ExternalInput":
            if nc.partition_id_tensor and name == nc.partition_id_tensor.name:
                continue
            in_names.append(name)
        elif alloc.kind == "ExternalOutput":
            out_names.append(name)
            out_avals.append(jax.core.ShapedArray(
                tuple(alloc.tensor_shape), _mb.dt.np(alloc.dtype)))
            zero_outs.append(np.zeros(tuple(alloc.tensor_shape),
                                      _mb.dt.np(alloc.dtype)))
    n_params = len(in_names)
    all_names = in_names + out_names

    pname = nc.partition_id_tensor.name if nc.partition_id_tensor else None

    def _body(*args):
        operands = list(args)
        names = list(all_names)
        if pname is not None:
            operands.append(bass2jax.partition_id_tensor())
            names.append(pname)
        outs = bass2jax._bass_exec_p.bind(
            *operands,
            out_avals=tuple(out_avals),
            in_names=tuple(names),
            out_names=tuple(out_names),
            lowering_input_output_aliases=(),
            sim_require_finite=True,
            sim_require_nnan=True,
            nc=nc,
        )
        return tuple(outs)

    devices = jax.devices()[:N_CORES]
    mesh = Mesh(np.asarray(devices), ("core",))
    nin = n_params + len(out_names)
    sharded = jax.jit(
        shard_map(_body, mesh=mesh,
                  in_specs=(PartitionSpec("core"),) * nin,
                  out_specs=(PartitionSpec("core"),) * len(out_names),
                  check_rep=False),
        keep_unused=True,
    )
    per_core = [[np.asarray(m[nm]) for nm in in_names] for m in in_maps]
    concat_in = [np.concatenate([per_core[c][i] for c in range(N_CORES)], axis=0)
                 for i in range(n_params)]
    concat_zeros = [np.zeros((N_CORES * z.shape[0], *z.shape[1:]), z.dtype)
                    for z in zero_outs]

    def post(out_arrs):
        full = np.asarray(out_arrs[0]).reshape(N_CORES, B_LOC, 256, N)
        return full.reshape(B, 256, N).reshape(B, 256, H, W).astype(np.float32)

    return sharded, concat_in, concat_zeros, post


if __name__ == "__main__":
    import reference as R
    import jax
    with jax.default_device(jax.devices('cpu')[0]):
        inputs = {k: np.asarray(v) for k, v in R.setup_inputs().items()}
    out = kernel(**inputs)
    print("out", out.shape, out.dtype)


# revision 57
# speedup vs baseline: 1.7433x; 1.1501x over previous
"""TRN2 Bass kernel for nn_Attention_59081570125142.

MobileViT-style attention block:
  qkv = BN(1x1conv(x)); per-head attention over N=1024 tokens
  (key_dim=16, head_dim=32, 8 heads); pos_enc = BN(dwconv3x3(v));
  out = BN(1x1conv(v_attn + pos_enc)).

Sharding: data-parallel over batch B=16 across 8 cores (2 images/core).
All BN folded into conv weights/biases on host. Matmuls in bf16,
accumulation fp32 in PSUM.

Softmax layout: logits computed KEY-major (logitsT[k, q] via lhsT=k_head,
rhs=q_head) so the softmax normalizer rides as a ones-column in the AV
matmul. The AV matmul is FLIPPED: est chunks [128k, 128q] are the
stationary lhsT and vts [128k, 33] streams as rhs, so the AV output land
QUERY-major ([q, head-dims]) and the denominator is a per-partition
column -> normalization is one strided reciprocal [128,8] plus one
broadcast multiply per group instead of per-column broadcast matmuls.
A cheap PE transpose (16x [128,128]) restores channel-major, with the
pos_enc add fused into the PSUM evacuation op.
QK packs 4 heads in the PE array via row tiling (K=16 each).
"""
import sys

sys.path.insert(0, '/opt/trn_rl_repo')

import numpy as np
import ml_dtypes

import concourse.bass as bass
import concourse.mybir as mybir
from concourse import tile
from concourse.bass_utils import run_bass_kernel_spmd

F32 = mybir.dt.float32
F32R = mybir.dt.float32r
BF16 = mybir.dt.bfloat16
AF = mybir.ActivationFunctionType
ALU = mybir.AluOpType

N_CORES = 8
B = 16
B_LOC = B // N_CORES          # 2 images per core
DIM = 256
NUM_HEADS = 8
HEAD_DIM = 32
KEY_DIM = 16
QKV_OUT = 512
N = 1024                      # 32*32 tokens
H = W = 32
SCALE = KEY_DIM ** -0.5       # 0.25
BN_EPS = 1e-3

_cache = {}


def _fold_bn(w, gamma, beta, mean, var):
    inv = gamma / np.sqrt(var + BN_EPS)
    return w * inv[:, None], beta - mean * inv


def _prep_weights(qkv_w, qkv_gamma, qkv_beta, qkv_mean, qkv_var,
                  pe_w, pe_gamma, pe_beta, pe_mean, pe_var,
                  proj_w, proj_gamma, proj_beta, proj_mean, proj_var):
    """Host-side BN folding + layout rearrangement."""
    W1, b1 = _fold_bn(qkv_w[:, :, 0, 0].astype(np.float32), qkv_gamma, qkv_beta,
                      qkv_mean, qkv_var)                      # (512, 256), (512,)
    W2, b2 = _fold_bn(proj_w[:, :, 0, 0].astype(np.float32), proj_gamma, proj_beta,
                      proj_mean, proj_var)                    # (256, 256), (256,)
    invpe = pe_gamma / np.sqrt(pe_var + BN_EPS)
    PW = (pe_w[:, 0] * invpe[:, None, None]).astype(np.float32)  # (256, 3, 3)
    bpe = (pe_beta - pe_mean * invpe).astype(np.float32)         # (256,)

    # channel-major qkv pass: out = W1T_cm.T @ x, 6 output tiles of 128:
    #  t0/t1: K heads 0-3 / 4-7 at 32-aligned slots (16 rows used per head)
    #  t2/t3: Q likewise;  t4/t5: V natural order (head h -> rows 32h..32h+32)
    w1t_cm = np.zeros((256, 768), np.float32)
    b1_cm = np.zeros((768,), np.float32)
    for h in range(NUM_HEADS):
        t = h // 4
        j = h % 4
        # K
        cols = t * 128 + 32 * j + np.arange(16)
        chans = h * 64 + 16 + np.arange(16)
        w1t_cm[:, cols] = W1[chans].T
        b1_cm[cols] = b1[chans]
        # Q
        cols = 256 + t * 128 + 32 * j + np.arange(16)
        chans = h * 64 + np.arange(16)
        w1t_cm[:, cols] = W1[chans].T
        b1_cm[cols] = b1[chans]
        # V
        cols = 512 + h * 32 + np.arange(32)
        chans = h * 64 + 32 + np.arange(32)
        w1t_cm[:, cols] = W1[chans].T
        b1_cm[cols] = b1[chans]

    # token-major v pass: vT[tok, 33h+d] = sum_c x[c,tok] * W1'[h*64+32+d, c];
    # col 33h+32 gets 0 from the matmul and 1.0 from the bias tile (ones col
    # for the fused softmax denominator).
    wvt = np.zeros((256, 264), np.float32)
    bvb = np.zeros((264,), np.float32)
    for h in range(NUM_HEADS):
        cols = 33 * h + np.arange(32)
        chans = h * 64 + 32 + np.arange(32)
        wvt[:, cols] = W1[chans].T
        bvb[cols] = b1[chans]
        bvb[33 * h + 32] = 1.0

    bf = ml_dtypes.bfloat16
    return dict(
        w1t=w1t_cm.astype(bf),                  # (256, 768)
        b1=b1_cm.reshape(768, 1).astype(np.float32),
        wvt=wvt.astype(bf),                     # (256, 264)
        bvb=np.broadcast_to(bvb, (128, 264)).copy().astype(np.float32),
        w2t=W2.T.copy().astype(bf),             # (256, 256)
        b2=b2.reshape(256, 1).astype(np.float32),
        pew=PW.reshape(256, 9).astype(np.float32),
        bpe=bpe.reshape(256, 1).astype(np.float32),
        ident=np.eye(128, dtype=bf),            # (128, 128) transpose identity
    )


def _split_waits(nc, cap=1, caps=None):
    """Walrus rejects instructions with more than ~1-2 semaphore waits
    ("Too many sync wait commands"). Hoist excess waits onto same-engine
    EventSemaphore carrier instructions inserted just before the offender —
    the engine stalls at the carriers first, semantically identical."""
    n = 0
    caps = caps or {}
    for f in nc.m.functions:
        for blk in f.blocks:
            insts = blk.instructions
            out = []
            for inst in insts:
                si = inst.sync_info
                waits = list(si.on_wait) if si else []
                # ACT executes strictly in order, so a wait on its OWN engine
                # semaphore (WAW guard for est-slot reuse) is redundant — but
                # it forces each exp to eat the previous one's write-ack
                # latency. Drop those.
                if (str(inst.engine) == "EngineType.Activation"
                        and waits and inst.opcode == "Activation"):
                    kept = [w for w in waits
                            if not str(w.ant_name).startswith("Activation")]
                    if len(kept) != len(waits):
                        waits = kept
                        si = mybir.SyncInfo(on_wait=waits,
                                            on_update=list(si.on_update))
                        inst.sync_info = si
                icap = caps.get(str(inst.opcode), cap)
                if len(waits) > icap and inst.opcode != "EventSemaphore":
                    extra, keep = waits[:-icap], waits[-icap:]
                    for k, w in enumerate(extra):
                        ev = mybir.InstEventSemaphore(
                            name=f"{inst.name}-sw{k}", ins=[], outs=[],
                            sync_info=mybir.SyncInfo(on_wait=[w], on_update=[]),
                        )
                        ev.engine = inst.engine
                        out.append(ev)
                        n += 1
                    inst.sync_info = mybir.SyncInfo(
                        on_wait=keep, on_update=list(si.on_update))
                out.append(inst)
            blk.instructions = out
    return n


def _build_program(reps=1):
    nc = bass.Bass()
    x_d = nc.declare_dram_parameter("x", [B_LOC, 256, N], BF16, isOutput=False)
    w1t_d = nc.declare_dram_parameter("w1t", [256, 768], BF16, isOutput=False)
    b1_d = nc.declare_dram_parameter("b1", [768, 1], F32, isOutput=False)
    wvt_d = nc.declare_dram_parameter("wvt", [256, 264], BF16, isOutput=False)
    bvb_d = nc.declare_dram_parameter("bvb", [128, 264], F32, isOutput=False)
    w2t_d = nc.declare_dram_parameter("w2t", [256, 256], BF16, isOutput=False)
    b2_d = nc.declare_dram_parameter("b2", [256, 1], F32, isOutput=False)
    pew_d = nc.declare_dram_parameter("pew", [256, 9], F32, isOutput=False)
    bpe_d = nc.declare_dram_parameter("bpe", [256, 1], F32, isOutput=False)
    ident_d = nc.declare_dram_parameter("ident", [128, 128], BF16, isOutput=False)
    out_d = nc.declare_dram_parameter("out", [B_LOC, 256, N], F32, isOutput=True)

    with tile.TileContext(nc) as tc:
        with (
            tc.tile_pool(name="const", bufs=1) as cpool,
            tc.tile_pool(name="xp", bufs=1) as xpool,
            tc.tile_pool(name="qkv", bufs=1) as qkvpool,
            tc.tile_pool(name="vt", bufs=1) as vtpool,
            tc.tile_pool(name="pe", bufs=1) as pepool,
            tc.tile_pool(name="petmp", bufs=3) as petmp,
            tc.tile_pool(name="exp", bufs=6) as exppool,
            tc.tile_pool(name="attn", bufs=1) as attnpool,
            tc.tile_pool(name="attnT", bufs=1) as attntpool,
            tc.tile_pool(name="nrm", bufs=3) as nrmpool,
            tc.tile_pool(name="outp", bufs=4) as outpool,
            tc.tile_pool(name="ps_small", bufs=1, space="PSUM") as ps_small, # 1 bank (qkv/vt/proj/transpose)
            tc.tile_pool(name="ps_qk", bufs=3, space="PSUM") as ps_qk,       # 6 banks
            tc.tile_pool(name="ps_av", bufs=1, space="PSUM") as ps_av,       # 1 bank
        ):
            # ---- load constants ----
            w1t = [cpool.tile([128, 768], BF16, tag=f"w1t{i}", name=f"w1t{i}") for i in range(2)]
            wvt = [cpool.tile([128, 264], BF16, tag=f"wvt{i}", name=f"wvt{i}") for i in range(2)]
            w2t = [cpool.tile([128, 256], BF16, tag=f"w2t{i}", name=f"w2t{i}") for i in range(2)]
            b2 = [cpool.tile([128, 1], F32, tag=f"b2{i}", name=f"b2{i}") for i in range(2)]
            pew = [cpool.tile([128, 9], F32, tag=f"pew{i}", name=f"pew{i}") for i in range(2)]
            bpe = [cpool.tile([128, 1], F32, tag=f"bpe{i}", name=f"bpe{i}") for i in range(2)]
            b1 = [cpool.tile([128, 1], F32, tag=f"b1{i}", name=f"b1{i}") for i in range(6)]
            bvb = cpool.tile([128, 264], F32, tag="bvb")
            ident = cpool.tile([128, 128], BF16, tag="ident", name="ident")
            xs = [[xpool.tile([128, N], BF16, tag=f"x{im}{ct}", name=f"x{im}{ct}") for ct in range(2)]
                  for im in range(B_LOC)]
            # DMA issue cost is ~0.5-0.8us each and serial per queue: put the
            # startup critical path (x img0, w1t, K/Q biases, vt weights)
            # first and split it across the two HWDGE queues (SP + ACT) so
            # the first QK->exp fires ~5us in, not ~19us.
            nc.sync.dma_start(xs[0][0][:], x_d[0, 0:128, :])
            nc.scalar.dma_start(xs[0][1][:], x_d[0, 128:256, :])
            nc.sync.dma_start(w1t[0][:], w1t_d[0:128, :])
            nc.scalar.dma_start(w1t[1][:], w1t_d[128:256, :])
            nc.sync.dma_start(b1[0][:], b1_d[0:128, :])
            nc.scalar.dma_start(b1[2][:], b1_d[256:384, :])
            nc.sync.dma_start(wvt[0][:], wvt_d[0:128, :])
            nc.scalar.dma_start(wvt[1][:], wvt_d[128:256, :])
            nc.sync.dma_start(bvb[:], bvb_d[:])
            for ot in (1, 3, 4, 5):
                nc.sync.dma_start(b1[ot][:], b1_d[128 * ot:128 * (ot + 1), :])
            for ct in range(2):
                nc.sync.dma_start(xs[1][ct][:], x_d[1, 128 * ct:128 * (ct + 1), :])
            for ct in range(2):
                nc.sync.dma_start(pew[ct][:], pew_d[128 * ct:128 * (ct + 1), :])
                nc.sync.dma_start(bpe[ct][:], bpe_d[128 * ct:128 * (ct + 1), :])
            nc.sync.dma_start(ident[:], ident_d[:])
            for ct in range(2):
                nc.sync.dma_start(w2t[ct][:], w2t_d[128 * ct:128 * (ct + 1), :])
                nc.sync.dma_start(b2[ct][:], b2_d[128 * ct:128 * (ct + 1), :])

            for _rep in range(reps):
                qkv_all = [[qkvpool.tile([128, N], BF16, tag=f"qkv{im}{ot}",
                                         name=f"qkv{im}{ot}") for ot in range(6)]
                           for im in range(B_LOC)]
                vts_all = [[vtpool.tile([128, 264], BF16, tag=f"vt{im}{tt}",
                                        name=f"vt{im}{tt}") for tt in range(8)]
                           for im in range(B_LOC)]
                attn_cm = [[attnpool.tile([128, N], BF16, tag=f"attn{im}{ct}",
                                          name=f"attn{im}{ct}")
                            for ct in range(2)] for im in range(B_LOC)]
                # query-major normalized attention, [q, 8 qchunks x 256 chans]
                attnT = [attntpool.tile([128, 2048], BF16, tag=f"attnT{im}",
                                        name=f"attnT{im}") for im in range(B_LOC)]
                peacc_all = [[None, None] for _ in range(B_LOC)]

                def emit_qkv_unit(im, ot, nq, alt=False):
                    # 256-token chunks keep each shared-psum-bank round trip
                    # (matmul + sem + DVE move) under the ~1.05us ACT period
                    # so the serialized unit chain never backs up into QK.
                    # alt=True borrows the (pre-attention) av bank so prefix
                    # units run as two parallel chains instead of one.
                    pool, tag = (ps_av, "av") if alt else (ps_small, "small")
                    pq = pool.tile([128, 256], F32, tag=tag, name="pq")
                    for ct in range(2):
                        nc.tensor.matmul(
                            pq[:], w1t[ct][:, 128 * ot:128 * (ot + 1)],
                            xs[im][ct][:, 256 * nq:256 * (nq + 1)],
                            start=(ct == 0), stop=(ct == 1))
                    nc.vector.tensor_scalar_add(
                        qkv_all[im][ot][:, 256 * nq:256 * (nq + 1)],
                        pq[:], b1[ot][:])

                def emit_vt_unit(im, tt):
                    pv = ps_small.tile([128, 512], F32, tag="small", name="pv")
                    for ct in range(2):
                        nc.tensor.matmul(
                            pv[:, 0:264],
                            xs[im][ct][:, 128 * tt:128 * (tt + 1)], wvt[ct][:],
                            start=(ct == 0), stop=(ct == 1))
                    nc.vector.tensor_tensor(vts_all[im][tt][:], pv[:, 0:264],
                                            bvb[:], op=ALU.add)

                def dwconv_ops(im):
                    # depthwise 3x3 on v (bf16), SAME-pad via interior sub-
                    # views (out-of-range contributions simply absent).
                    # Center tap: tensor_scalar (4x DVE mode) initializes acc
                    # incl. bias. Taps 1-4 on DVE as premult (tensor_scalar,
                    # 4x) + in-place shifted add (tensor_tensor, 2x). Taps
                    # 5-8 on the otherwise-idle GPSIMD engine (in-place
                    # scalar_tensor_tensor), emitted with the last DVE tap so
                    # they claim no filler slots.
                    ops = []
                    for ct in range(2):
                        acc = pepool.tile([128, N], BF16,
                                          tag=f"peacc{im}{ct}",
                                          name=f"peacc{im}{ct}")
                        a2d = acc[:].rearrange("p (h w) -> p h w", h=H)
                        vimg = qkv_all[im][4 + ct][:].rearrange(
                            "p (h w) -> p h w", h=H)

                        def mkcenter(a2d=a2d, vimg=vimg, ct=ct):
                            nc.vector.tensor_scalar(
                                a2d, vimg, pew[ct][:, 4:5],
                                bpe[ct][:], op0=ALU.mult, op1=ALU.add)

                        ops.append(mkcenter)
                        taps = [(ky, kx) for ky in range(3) for kx in range(3)
                                if not (ky == 1 and kx == 1)]

                        def geom(ky, kx):
                            oy0, oy1 = max(0, 1 - ky), 32 - max(0, ky - 1)
                            ox0, ox1 = max(0, 1 - kx), 32 - max(0, kx - 1)
                            return (oy0, oy1, ox0, ox1,
                                    oy0 + ky - 1, oy1 + ky - 1,
                                    ox0 + kx - 1, ox1 + kx - 1)

                        def dve_tap(ky, kx, ct=ct, a2d=a2d, vimg=vimg):
                            oy0, oy1, ox0, ox1, iy0, iy1, ix0, ix1 = \
                                geom(ky, kx)
                            tmp = petmp.tile([128, N], BF16, tag="dwtmp",
                                             name="dwtmp")
                            t2d = tmp[:].rearrange("p (h w) -> p h w", h=H)
                            nc.vector.tensor_scalar(
                                tmp[:], qkv_all[im][4 + ct][:],
                                pew[ct][:, (3 * ky + kx):(3 * ky + kx) + 1],
                                None, op0=ALU.mult)
                            nc.vector.tensor_tensor(
                                a2d[:, oy0:oy1, ox0:ox1],
                                a2d[:, oy0:oy1, ox0:ox1],
                                t2d[:, iy0:iy1, ix0:ix1], op=ALU.add)

                        def pool_tap(ky, kx, ct=ct, a2d=a2d, vimg=vimg):
                            # GPSIMD's ISA has no scalar_tensor_tensor: DVE
                            # premultiplies (tensor_scalar, 4x mode) and the
                            # Pool engine does the in-place shifted add
                            oy0, oy1, ox0, ox1, iy0, iy1, ix0, ix1 = \
                                geom(ky, kx)
                            tmp = petmp.tile([128, N], BF16, tag="pooltmp",
                                             name="pooltmp", bufs=4)
                            t2d = tmp[:].rearrange("p (h w) -> p h w", h=H)
                            nc.vector.tensor_scalar(
                                tmp[:], qkv_all[im][4 + ct][:],
                                pew[ct][:, (3 * ky + kx):(3 * ky + kx) + 1],
                                None, op0=ALU.mult)
                            nc.gpsimd.tensor_tensor(
                                a2d[:, oy0:oy1, ox0:ox1],
                                a2d[:, oy0:oy1, ox0:ox1],
                                t2d[:, iy0:iy1, ix0:ix1], op=ALU.add)

                        for k in range(4):
                            ops.append(lambda t=taps[k], f=dve_tap: f(*t))
                        for k in range(4, 8):
                            ops.append(lambda t=taps[k], g=pool_tap: g(*t))
                        peacc_all[im][ct] = acc
                    return ops

                ots_all = [{} for _ in range(B_LOC)]

                def post_half(im, h):
                    # one qc-half (512 queries): transposes back to channel-
                    # major + fused pos_enc add + proj + out DMA, as
                    # single-op closures for filler slots
                    ops = []
                    ptrs = {}

                    def mktr(ct, part):
                        def f():
                            if part == 0:
                                ptrs[ct] = ps_small.tile(
                                    [128, 512], BF16, tag="small", name="ptr")
                            for a in (4 * h + 2 * part, 4 * h + 2 * part + 1):
                                nc.tensor.transpose(
                                    ptrs[ct][:, 128 * (a - 4 * h):
                                             128 * (a - 4 * h) + 128],
                                    attnT[im][:, 256 * a + 128 * ct:
                                              256 * a + 128 * ct + 128],
                                    ident[:])
                        return f

                    def mkevac(ct):
                        def f():
                            nc.vector.tensor_tensor(
                                attn_cm[im][ct][:, 512 * h:512 * h + 512],
                                ptrs.pop(ct)[:],
                                peacc_all[im][ct][:, 512 * h:512 * h + 512],
                                op=ALU.add)
                        return f

                    def mkproj(ot):
                        def f():
                            ots = ots_all[im]
                            if ot not in ots:
                                ots[ot] = outpool.tile([128, N], F32,
                                                       tag="osb", name="ot_sb")
                            pp = ps_small.tile([128, 512], F32, tag="small",
                                               name="pp")
                            for ct in range(2):
                                nc.tensor.matmul(
                                    pp[:], w2t[ct][:, 128 * ot:128 * (ot + 1)],
                                    attn_cm[im][ct][:, 512 * h:512 * h + 512],
                                    start=(ct == 0), stop=(ct == 1))
                            nc.vector.tensor_scalar_add(
                                ots[ot][:, 512 * h:512 * h + 512], pp[:],
                                b2[ot][:])
                            nc.sync.dma_start(
                                out_d[im, 128 * ot:128 * (ot + 1),
                                      512 * h:512 * h + 512],
                                ots[ot][:, 512 * h:512 * h + 512])
                        return f

                    ops += [mktr(0, 0), mktr(0, 1), mkevac(0),
                            mktr(1, 0), mktr(1, 1), mkevac(1)]
                    ops += [mkproj(0), mkproj(1)]
                    return ops

                # ---- minimal img0 prefix: what (qc0, g0/g1) needs first.
                # Units alternate between the small/av banks (two parallel
                # chains); the first QK is emitted as soon as its three
                # producer units are, so the PE reaches it early.

                # Everything else (rest of img0 preproc, both dwconvs, all of
                # img1 preproc) is injected into the attention pipeline, ONE
                # closure per kt slot, ordered so each unit's EMISSION
                # precedes its first reader's emission (the dep tracker only
                # sees program order).
                def q_unit(im, ot, nq):
                    return lambda: emit_qkv_unit(im, ot, nq)

                def v_unit(im, tt):
                    return lambda: emit_vt_unit(im, tt)

                fillers = []
                fillers += [v_unit(0, 2), v_unit(0, 3), q_unit(0, 0, 3),
                            v_unit(0, 4), v_unit(0, 5), v_unit(0, 6),
                            v_unit(0, 7)]
                fillers += [q_unit(0, 1, 0), q_unit(0, 3, 0), q_unit(0, 3, 1),
                            q_unit(0, 1, 1), q_unit(0, 1, 2), q_unit(0, 1, 3)]
                fillers += [q_unit(0, 2, 2), q_unit(0, 2, 3),
                            q_unit(0, 3, 2), q_unit(0, 3, 3)]
                fillers += [q_unit(0, ot, nq) for ot in (4, 5)
                            for nq in range(4)]
                if B_LOC > 1:
                    # im1's boundary-critical units go ahead of im0's dwconv:
                    # the shared psum bank serializes preproc units, so they
                    # need a long runway before i==8 consumes them
                    fillers += [q_unit(1, 0, 0), q_unit(1, 2, 0),
                                q_unit(1, 2, 1), v_unit(1, 0),
                                q_unit(1, 0, 1), v_unit(1, 1),
                                v_unit(1, 2), v_unit(1, 3)]
                fillers += dwconv_ops(0)
                if B_LOC > 1:
                    fillers += [v_unit(1, tt) for tt in range(4, 8)]
                    fillers += [q_unit(1, 0, 2), q_unit(1, 0, 3),
                                q_unit(1, 2, 2), q_unit(1, 2, 3)]
                    fillers += [q_unit(1, 1, 0), q_unit(1, 1, 1),
                                q_unit(1, 3, 0), q_unit(1, 3, 1)]
                    fillers += [q_unit(1, 1, 2), q_unit(1, 1, 3),
                                q_unit(1, 3, 2), q_unit(1, 3, 3)]
                    fillers += [q_unit(1, ot, nq) for ot in (4, 5)
                                for nq in range(4)]
                    fillers += dwconv_ops(1)

                # ---- attention, software-pipelined across all groups ----
                # 2-head groups g: heads A=2g, B=2g+1 share K/Q tile t=g//2.
                # QK row-tiled (K=16 at 32-aligned rows), one PSUM BANK per
                # head. AV is flipped (est stationary, vts moving): out
                # [128 queries, (4 qchunks x 2 heads x 33)] in one bank.
                # QK(kt+1) issues before AV(kt) so the in-order PE never
                # stalls ACT; each group's normalization is deferred into the
                # next group's kt loop.
                # qc-major per image: all groups' qc0 first, so the first
                # half's post work (transpose+proj) can overlap the second
                # half's attention
                seq = [(im, g, qc) for im in range(B_LOC)
                       for qc in range(2) for g in range(4)]
                pqk_store = {}

                def emit_qk(i, kt):
                    im, g, qc = seq[i]
                    t = g // 2
                    jA = (2 * g) % 4
                    pqk = ps_qk.tile([128, 1024], F32, tag="qk", name="pqk")
                    for j, bank in ((jA, 0), (jA + 1, 1)):
                        nc.tensor.matmul(
                            pqk[:, 512 * bank:512 * (bank + 1)],
                            qkv_all[im][t][32 * j:32 * j + 16,
                                           128 * kt:128 * (kt + 1)],
                            qkv_all[im][2 + t][32 * j:32 * j + 16,
                                               512 * qc:512 * (qc + 1)],
                            tile_position=(32 * j, 0))
                    pqk_store[(i, kt)] = pqk

                def emit_norm(i, pav):
                    im, g, qc = seq[i]
                    rz = nrmpool.tile([128, 8], BF16, tag="rz", name="rz")
                    pav3 = pav.rearrange("p (c hp d) -> p c hp d", c=4, d=33)
                    rz3 = rz[:].rearrange("p (c hp) -> p c hp", c=4)
                    with nc.allow_low_precision(reason="bf16 softmax den"):
                        nc.vector.reciprocal(rz3, pav3[:, :, :, 32])
                    outap = attnT[im][:].rearrange(
                        "p (a g hp d) -> p a g hp d", g=4, hp=2, d=32)[
                        :, 4 * qc:4 * qc + 4, g]
                    rzb = rz3.unsqueeze(3).broadcast_to([128, 4, 2, 32])
                    nc.vector.tensor_tensor(outap, pav3[:, :, :, 0:32], rzb,
                                            op=ALU.mult)

                # Schraudolph fast-exp on DVE for ~1/5 of the tiles to
                # offload the bottleneck ACT engine: est = bitcast<bf16>(
                # int16(round(logit * SCALE*128/ln2 + 16256))). ~3% per-
                # element error that largely cancels in the softmax ratio.
                SCHR_A = float(SCALE * 128.0 / np.log(2.0))
                SCHR_B = 16256.0

                def schr(i, kt):
                    # the DVE est op gates QK(kt+3) via the psum rotation, so
                    # keep it off the i-ranges where the DVE queue is flooded
                    # by dwconv fillers (i 4-5, 11-12) and the im1 preproc
                    # catch-up (i 8)
                    if i in (4, 5, 8, 12, 13):
                        return False
                    if i == 15:
                        # kt7 stays on ACT: the est->AV->norm chain of the
                        # very last group is the tail's critical path on DVE
                        return kt % 2 == 1 and kt < 7
                    if i in (9, 10) and kt == 0:
                        return True
                    return (8 * i + kt) % 3 == 2

                emit_qkv_unit(0, 0, 0)             # K nq0
                emit_qkv_unit(0, 2, 0, alt=True)   # Q nq0
                emit_qkv_unit(0, 2, 1)             # Q nq1
                emit_qk(0, 0)
                emit_qkv_unit(0, 0, 1, alt=True)   # K nq1
                emit_vt_unit(0, 0)
                emit_qkv_unit(0, 0, 2, alt=True)   # K nq2
                emit_vt_unit(0, 1)
                norm_pending = []
                post_fillers = []
                tail_rest = []
                for i in range(len(seq)):
                    im, g, qc = seq[i]
                    hA, hB = 2 * g, 2 * g + 1
                    if norm_pending:
                        # norm(i-1) before pav(i): with ps_av bufs=1 the
                        # AV(i) matmuls reuse the bank norm(i-1) reads
                        emit_norm(*norm_pending.pop(0))
                    if i in (7, 9, 14):
                        # previous qc-half fully normalized; queue its
                        # transposes+proj ahead of other fillers (h0 posts
                        # wait extra groups so the dwconv tail is emitted
                        # before the evac that reads peacc)
                        pim, ph = {7: (0, 0), 9: (0, 1), 14: (1, 0)}[i]
                        post_fillers += post_half(pim, ph)
                    if i == 15:
                        # h11's ct0 transposes+evac only need g0/g1 norms
                        # (done); start them inside the loop, finish at tail
                        tail_ops = post_half(1, 1) if B_LOC > 1 \
                            else post_half(0, 1)
                        post_fillers += tail_ops[:3]
                        tail_rest = tail_ops[3:]
                    pav = ps_av.tile([128, 264], F32, tag="av", name="pav")
                    for kt in range(8):
                        est = exppool.tile([128, 1024], BF16, tag="est",
                                           name="est")
                        pqk = pqk_store.pop((i, kt))
                        if schr(i, kt):
                            nc.vector.tensor_scalar(
                                est[:].bitcast(mybir.dt.int16), pqk[:],
                                SCHR_A, SCHR_B, op0=ALU.mult, op1=ALU.add)
                        else:
                            nc.scalar.activation(est[:], pqk[:],
                                                 AF.Exp, scale=SCALE)
                        if kt + 1 < 8:
                            emit_qk(i, kt + 1)
                        elif i + 1 < len(seq):
                            emit_qk(i + 1, 0)
                        # start=True only on the FIRST matmul of the bank:
                        # on HW (and in CoreSim) start marks the whole 2KB
                        # zero-region pending-zero, so later start=False
                        # writes overwrite-where-pending and accumulate after.
                        for c in range(4):
                            for hp in range(2):
                                nc.tensor.matmul(
                                    pav[:, 66 * c + 33 * hp:
                                        66 * c + 33 * hp + 33],
                                    est[:, 512 * hp + 128 * c:
                                        512 * hp + 128 * c + 128],
                                    vts_all[im][kt][:, 33 * (2 * g + hp):
                                                    33 * (2 * g + hp) + 33],
                                    start=(kt == 0 and c == 0 and hp == 0),
                                    stop=(kt == 7),
                                    skip_group_check=True)
                        if post_fillers and (kt > 2 or i not in (7, 9, 14)):
                            post_fillers.pop(0)()
                        elif fillers:
                            fillers.pop(0)()
                    norm_pending.append((i, pav))
                emit_norm(*norm_pending.pop(0))
                for op in fillers:
                    op()
                for op in post_fillers:
                    op()
                for op in tail_rest:
                    op()
    _split_waits(nc)
    return nc


def kernel(**inputs):
    x = np.asarray(inputs['x'], np.float32)
    Bful, C, Hh, Ww = x.shape
    assert (Bful, C, Hh, Ww) == (B, DIM, H, W)

    key = 'prog1'
    if key not in _cache:
        _cache[key] = _build_program()
    nc = _cache[key]

    wd = _prep_weights(
        np.asarray(inputs['qkv_w'], np.float32), np.asarray(inputs['qkv_gamma'], np.float32),
        np.asarray(inputs['qkv_beta'], np.float32), np.asarray(inputs['qkv_mean'], np.float32),
        np.asarray(inputs['qkv_var'], np.float32),
        np.asarray(inputs['pe_w'], np.float32), np.asarray(inputs['pe_gamma'], np.float32),
        np.asarray(inputs['pe_beta'], np.float32), np.asarray(inputs['pe_mean'], np.float32),
        np.asarray(inputs['pe_var'], np.float32),
        np.asarray(inputs['proj_w'], np.float32), np.asarray(inputs['proj_gamma'], np.float32),
        np.asarray(inputs['proj_beta'], np.float32), np.asarray(inputs['proj_mean'], np.float32),
        np.asarray(inputs['proj_var'], np.float32),
    )

    xr = x.reshape(B, 256, N).astype(ml_dtypes.bfloat16)
    in_maps = []
    for c in range(N_CORES):
        m = dict(wd)
        m['x'] = xr[B_LOC * c:B_LOC * (c + 1)]
        in_maps.append(m)

    res = run_bass_kernel_spmd(nc, in_maps, list(range(N_CORES)))
    out = np.concatenate([r['out'] for r in res.results], axis=0)
    return out.reshape(B, 256, H, W).astype(np.float32)


def make_runner(**inputs):
    """Build (jitted_fn, concat_inputs, zero_outs, postprocess) for benchmarking.

    Mirrors bass2jax.run_bass_via_pjrt's multi-core path but without donation
    so the same buffers can be re-executed for timing."""
    import jax
    from jax.sharding import Mesh, PartitionSpec
    from jax.experimental.shard_map import shard_map
    from concourse import bass2jax, mybir as _mb

    x = np.asarray(inputs['x'], np.float32)
    wd = _prep_weights(**{k: np.asarray(inputs[k], np.float32) for k in (
        'qkv_w', 'qkv_gamma', 'qkv_beta', 'qkv_mean', 'qkv_var',
        'pe_w', 'pe_gamma', 'pe_beta', 'pe_mean', 'pe_var',
        'proj_w', 'proj_gamma', 'proj_beta', 'proj_mean', 'proj_var')})
    reps = int(inputs.get('_bench_reps', 1))
    ck = f'prog{reps}'
    if ck not in _cache:
        _cache[ck] = _build_program(reps)
    nc = _cache[ck]
    xr = x.reshape(B, 256, N).astype(ml_dtypes.bfloat16)
    in_maps = []
    for c in range(N_CORES):
        m = dict(wd)
        m['x'] = xr[B_LOC * c:B_LOC * (c + 1)]
        in_maps.append(m)

    bass2jax.install_neuronx_cc_hook()
    in_names, out_names, out_avals, zero_outs = [], [], [], []
    for alloc in nc.m.functions[0].allocations:
        if not isinstance(alloc, _mb.MemoryLocationSet):
            continue
        name = alloc.memorylocations[0].name
        if alloc.kind == "ExternalInput":
            if nc.partition_id_tensor and name == nc.partition_id_tensor.name:
                continue
            in_names.append(name)
        elif alloc.kind == "ExternalOutput":
            out_names.append(name)
            out_avals.append(jax.core.ShapedArray(
                tuple(alloc.tensor_shape), _mb.dt.np(alloc.dtype)))
            zero_outs.append(np.zeros(tuple(alloc.tensor_shape),
                                      _mb.dt.np(alloc.dtype)))
    n_params = len(in_names)
    all_names = in_names + out_names

    pname = nc.partition_id_tensor.name if nc.partition_id_tensor else None

    def _body(*args):
        operands = list(args)
        names = list(all_names)
        if pname is not None:
            operands.append(bass2jax.partition_id_tensor())
            names.append(pname)
        outs = bass2jax._bass_exec_p.bind(
            *operands,
            out_avals=tuple(out_avals),
            in_names=tuple(names),
            out_names=tuple(out_names),
            lowering_input_output_aliases=(),
            sim_require_finite=True,
            sim_require_nnan=True,
            nc=nc,
        )
        return tuple(outs)

    devices = jax.devices()[:N_CORES]
    mesh = Mesh(np.asarray(devices), ("core",))
    nin = n_params + len(out_names)
    sharded = jax.jit(
        shard_map(_body, mesh=mesh,
                  in_specs=(PartitionSpec("core"),) * nin,
                  out_specs=(PartitionSpec("core"),) * len(out_names),
                  check_rep=False),
        keep_unused=True,
    )
    per_core = [[np.asarray(m[nm]) for nm in in_names] for m in in_maps]
    concat_in = [np.concatenate([per_core[c][i] for c in range(N_CORES)], axis=0)
                 for i in range(n_params)]
    concat_zeros = [np.zeros((N_CORES * z.shape[0], *z.shape[1:]), z.dtype)
                    for z in zero_outs]

    def post(out_arrs):
        full = np.asarray(out_arrs[0]).reshape(N_CORES, B_LOC, 256, N)
        return full.reshape(B, 256, N).reshape(B, 256, H, W).astype(np.float32)

    return sharded, concat_in, concat_zeros, post


if __name__ == "__main__":
    import reference as R
    import jax
    with jax.default_device(jax.devices('cpu')[0]):
        inputs = {k: np.asarray(v) for k, v in R.setup_inputs().items()}
    out = kernel(**inputs)
    print("out", out.shape, out.dtype)
